# revision 30
# baseline (speedup 1.0000x reference)
"""DeepReservoir (2-layer leaky ESN) Trainium2 kernel — time-split, PSUM-u.

Reference computation (per layer):
    u = x @ K + b
    h_t = 0.1*h_{t-1} + 0.9*tanh(u_t + h_{t-1} @ W)
Layer 1 consumes layer 0's states; output = concat(s0, s1) on features.

Sharding: per-step cost is dominated by streaming the recurrent weights
into the PE array, independent of batch — so TIME-SPLIT, not batch
split: core c computes segment t in [c*128, (c+1)*128) for the full
batch (B=32), warm-started 32 steps earlier (the reservoir is
contractive; warm-up error ~1e-6). A per-core {0,1} mask zeroes the
state at the boundary so core 0's segment is exact from h=0.

Critical-path design: the projection u is accumulated IN PSUM — per
quarter (8 steps), a bias-seed matmul (rank-4: bias block times an
indicator) writes each PSUM bank once with start=True, the projection
GEMMs accumulate, and the per-step recurrent matmuls accumulate on
top (start=False; start=True clears a whole bank's has_written), so
tanh reads PSUM directly and the per-step chain is MM -> tanh -> leaky
update (no separate add). GEMM matmuls are drip-fed between step
matmuls to fill the PE stalls left by the chain latency. Layer 1 runs
one quarter behind layer 0. State stored scaled (htil = h/0.9, scales
folded into weights, final 0.9 rescale on host); matmuls fp16.
"""
import sys
import types

import numpy as np

B, T, I, U = 32, 1024, 64, 512     # full batch per core (time-split)
NCORES = 8
SEG = T // NCORES                  # 128 segment steps per core
LEAD = 16                          # warm-up steps (1 chunk)
Tc = 16                            # chunk length (state/DMA granularity)
QT = 8                             # steps per quarter (PSUM-u granularity)
NC = (LEAD + SEG) // Tc            # 9 chunks per core
NQ = NC * Tc // QT                 # 18 quarters
ALPHA = 0.9

_COMPILED = {}


# ---------------------------------------------------------------------------
# environment patches (inlined so kernel.py is self-contained)
# ---------------------------------------------------------------------------
def _apply_patches():
    import concourse.tile as tilemod
    from concourse.vector_clock import ScopedClock

    if not getattr(tilemod.TileContext, "_drain_patch_applied", False):
        def _drain_and_barrier(self, tick_clock, wait_clock):
            nc = self.nc
            drain_inst = nc.sync.drain()
            wait_clock.add_sem_waits(
                drain_inst.ins, ScopedClock({None: tick_clock.global_clock})
            )
            waits = list(drain_inst.ins.sync_info.on_wait)
            if len(waits) > 1:
                drain_inst.ins.sync_info.on_wait = waits[:1]
                for w in waits[1:]:
                    extra = nc.sync.drain()
                    si = extra.ins.sync_info
                    if si is None:
                        import bass_rust
                        extra.ins.sync_info = bass_rust.SyncInfo(
                            on_wait=[w], on_update=[]
                        )
                    else:
                        si.on_wait = [w]
            nc.all_engine_barrier()
            assert self.sems is not None
            popped = nc._tile_sem_poison_stack.pop()
            assert popped is self._sem_poison
            nc.clear_and_free_semaphores(list(self.sems.allocated().values()))
            nc.all_engine_barrier()

        tilemod.TileContext._drain_and_barrier = _drain_and_barrier
        tilemod.TileContext._drain_patch_applied = True

    import antenv
    if not hasattr(antenv, "axon_hooks"):
        mod = types.ModuleType("antenv.axon_hooks")
        mod._hook = None
        mod.set_axon_ntff_profile_hook = lambda h: setattr(mod, "_hook", h)
        mod.get_axon_ntff_profile_hook = lambda: mod._hook
        sys.modules["antenv.axon_hooks"] = mod
        antenv.axon_hooks = mod
        try:
            from trn_agent_boot.trn_boot import _ntff_profile_via_ctypes
            hook = _ntff_profile_via_ctypes("/opt/axon/libaxon_pjrt.so")
            if hook is not None:
                mod.set_axon_ntff_profile_hook(hook)
        except Exception:
            pass


def _split_sync_waits(nc, max_waits=1):
    """The public walrus rejects instructions with >2 sync-wait commands.
    Spread overflow waits onto same-engine NOPs inserted just before."""
    import concourse.mybir as mybir

    for f in nc.m.functions:
        for blk in f.blocks:
            insts = blk.instructions
            out = []
            changed = False
            for inst in insts:
                si = getattr(inst, "sync_info", None)
                waits = list(si.on_wait) if si is not None else []
                if len(waits) > max_waits:
                    changed = True
                    overflow = waits[:-max_waits]
                    si.on_wait = waits[-max_waits:]
                    for i in range(0, len(overflow), max_waits):
                        nop = mybir.InstNoOp(
                            name=nc.get_next_instruction_name(),
                            sync_info=mybir.SyncInfo(
                                on_wait=overflow[i:i + max_waits], on_update=[]
                            ),
                            bass_nofuse=True,
                            engine=inst.engine,
                        )
                        out.append(nop)
                out.append(inst)
            if changed:
                blk.instructions = out
    return nc


# ---------------------------------------------------------------------------
# kernel builder
# ---------------------------------------------------------------------------
def build_nc():
    import concourse.bass as bass
    import concourse.tile as tile
    import concourse.mybir as mybir

    f32 = mybir.dt.float32
    f16 = mybir.dt.float16
    S = 4 * B                  # free cols per step (4 U-tiles x B) = 128
    SLAB = Tc * B              # cols per (chunk, feature-tile) slab = 1024
    QS = QT * B                # cols per (quarter, feature-tile) slab = 256
    TL = NC * Tc               # local timeline length = 160

    nc = bass.Bass(trn_type="TRN2")

    xT_d = nc.declare_dram_parameter("xT", (I, TL * B), f16, isOutput=False)
    k0_d = nc.declare_dram_parameter("k0", (I, U), f16, isOutput=False)
    w0_d = nc.declare_dram_parameter("w0", (128, 4 * U), f16, isOutput=False)
    k1_d = nc.declare_dram_parameter("k1", (128, 4 * U), f16, isOutput=False)
    w1_d = nc.declare_dram_parameter("w1", (128, 4 * U), f16, isOutput=False)
    b0_d = nc.declare_dram_parameter("b0blk", (4, 128), f16, isOutput=False)
    bb_d = nc.declare_dram_parameter("b1blk", (4, 128), f16, isOutput=False)
    in_d = nc.declare_dram_parameter("ind", (4, 4 * QS), f16, isOutput=False)
    mk_d = nc.declare_dram_parameter("mask", (128, S), f16, isOutput=False)
    h0_d = nc.declare_dram_parameter("h0T", (U, SEG * B), f16, isOutput=True)
    h1_d = nc.declare_dram_parameter("h1T", (U, SEG * B), f16, isOutput=True)

    with tile.TileContext(nc) as tc:
        import contextlib
        with contextlib.ExitStack() as ctx:
            const = ctx.enter_context(tc.tile_pool(name="const", bufs=1))
            ypool = ctx.enter_context(tc.tile_pool(name="ypool", bufs=3))
            up0 = ctx.enter_context(
                tc.tile_pool(name="up0", bufs=2, space="PSUM"))
            up1 = ctx.enter_context(
                tc.tile_pool(name="up1", bufs=2, space="PSUM"))

            # --- resident constants: startup DMAs spread across engine DGE
            # queues so they run in parallel rings; the critical first-step
            # set (k0/b0blk/ind/w0/xT head) gets its own queues ------------
            k0 = const.tile([I, U], f16, tag="k0")
            nc.sync.dma_start(k0[:], k0_d[:, :])
            b0blk = const.tile([4, 128], f16, tag="b0blk")
            nc.sync.dma_start(b0blk[:], b0_d[:, :])
            ind = const.tile([4, 4 * QS], f16, tag="ind")
            nc.sync.dma_start(ind[:], in_d[:, :])
            w0 = const.tile([128, 4 * U], f16, tag="w0")
            nc.scalar.dma_start(w0[:, :2 * U], w0_d[:, :2 * U])
            nc.gpsimd.dma_start(w0[:, 2 * U:], w0_d[:, 2 * U:])
            xT = const.tile([I, TL * B], f16, tag="xT")
            XSPLIT = 4 * QS
            nc.sync.dma_start(xT[:, :XSPLIT], xT_d[:, :XSPLIT])
            b1blk = const.tile([4, 128], f16, tag="b1blk")
            nc.sync.dma_start(b1blk[:], bb_d[:, :])
            k1 = const.tile([128, 4 * U], f16, tag="k1")
            nc.scalar.dma_start(k1[:], k1_d[:, :])
            w1 = const.tile([128, 4 * U], f16, tag="w1")
            nc.gpsimd.dma_start(w1[:], w1_d[:, :])
            mask = const.tile([128, S], f16, tag="mask")
            nc.sync.dma_start(mask[:], mk_d[:, :])
            nc.sync.dma_start(xT[:, XSPLIT:], xT_d[:, XSPLIT:])
            zero = const.tile([128, S], f16, tag="zero")
            nc.vector.memset(zero[:], 0.0)

            # state chunks, double-buffered (slab layout:
            # col = j*SLAB + t*B + b, j = feature 128-tile)
            hist0 = [const.tile([128, Tc * S], f16, tag=f"hist0_{i}", name=f"hist0_{i}") for i in range(2)]
            hist1 = [const.tile([128, Tc * S], f16, tag=f"hist1_{i}", name=f"hist1_{i}") for i in range(2)]

            # per-quarter PSUM u tiles (col = m*QS + r*B + b), keyed by quarter
            u0_tiles = {}
            u1_tiles = {}

            def mm_rhs_ap(hist_bufs, t, j):
                """[128, B] moving operand: state k-tile j at step t."""
                if t < 0:
                    return zero[:, j * B:(j + 1) * B]
                return hist_bufs[(t // Tc) % 2][
                    :, j * SLAB + (t % Tc) * B: j * SLAB + (t % Tc + 1) * B]

            def state_xs(hist_bufs, t):
                """[128, 4, B] strided cross-section of the state at step t."""
                if t < 0:
                    return zero[:, :].rearrange("p (j b) -> p j b", j=4)
                buf = hist_bufs[(t // Tc) % 2]
                r = t % Tc
                return buf[:, :].rearrange(
                    "p (j tb) -> p j tb", j=4)[:, :, r * B:(r + 1) * B]

            def u0_emit(q):
                """Closures: u0 quarter q = bias seed + x-projection.
                Each PSUM bank gets exactly ONE start=True write (the seed);
                start=True clears the whole bank's has_written bits, so any
                later start on a bank would break accumulation into it."""
                tl = up0.tile([128, QT * S], f32, tag="u0q", name="u0q")
                u0_tiles[q] = tl
                cols = slice(q * QS, (q + 1) * QS)
                ops = [
                    (lambda hh=hh, tl=tl: nc.tensor.matmul(
                        tl[:, hh * 512:(hh + 1) * 512], lhsT=b0blk[:, :],
                        rhs=ind[:, hh * 512:(hh + 1) * 512],
                        start=True, stop=False, skip_group_check=True))
                    for hh in range(2)
                ]
                ops += [
                    (lambda m=m, tl=tl: nc.tensor.matmul(
                        tl[:, m * QS:(m + 1) * QS],
                        lhsT=k0[:, m * 128:(m + 1) * 128], rhs=xT[:, cols],
                        start=False, stop=False, skip_group_check=True))
                    for m in range(4)
                ]
                return ops

            def u1_emit(q):
                """Closures: u1 quarter q = bias seed + hist0-projection."""
                tl = up1.tile([128, QT * S], f32, tag="u1q", name="u1q")
                u1_tiles[q] = tl
                src = hist0[(q * QT // Tc) % 2]
                qo = (q % (Tc // QT)) * QS
                ops = [
                    (lambda hh=hh, tl=tl: nc.tensor.matmul(
                        tl[:, hh * 512:(hh + 1) * 512], lhsT=b1blk[:, :],
                        rhs=ind[:, hh * 512:(hh + 1) * 512],
                        start=True, stop=False, skip_group_check=True))
                    for hh in range(2)
                ]
                for m in range(4):
                    for k in range(4):
                        ops.append(lambda m=m, k=k, tl=tl: nc.tensor.matmul(
                            tl[:, m * QS:(m + 1) * QS],
                            lhsT=k1[:, k * U + m * 128: k * U + (m + 1) * 128],
                            rhs=src[:, k * SLAB + qo: k * SLAB + qo + QS],
                            start=False, stop=False, skip_group_check=True))
                return ops

            def step(layer, t):
                w, utiles, hist = (
                    (w0, u0_tiles, hist0) if layer == 0 else
                    (w1, u1_tiles, hist1))
                tl = utiles[t // QT]
                r8 = t % QT
                for m in range(4):
                    for k in range(4):
                        nc.tensor.matmul(
                            tl[:, m * QS + r8 * B: m * QS + (r8 + 1) * B],
                            lhsT=w[:, k * U + m * 128: k * U + (m + 1) * 128],
                            rhs=mm_rhs_ap(hist, t - 1, k),
                            start=False, stop=False, skip_group_check=True)
                u_xs = tl[:, :].rearrange(
                    "p (m tb) -> p m tb", m=4)[:, :, r8 * B:(r8 + 1) * B]
                y = ypool.tile([128, S], f16, tag="y", name="y")
                nc.scalar.activation(
                    y[:].rearrange("p (m b) -> p m b", m=4), u_xs,
                    mybir.ActivationFunctionType.Tanh)
                nc.vector.scalar_tensor_tensor(
                    state_xs(hist, t), state_xs(hist, t - 1), 0.1,
                    y[:].rearrange("p (j b) -> p j b", j=4),
                    op0=mybir.AluOpType.mult, op1=mybir.AluOpType.add)
                if t == LEAD - 1:
                    # warm-up/segment boundary: core 0 (mask=0) starts its
                    # segment from the exact h=0 initial state
                    nc.vector.scalar_tensor_tensor(
                        state_xs(hist, t), state_xs(hist, t), 1.0,
                        mask[:, :].rearrange("p (j b) -> p j b", j=4),
                        op0=mybir.AluOpType.mult, op1=mybir.AluOpType.mult)

            def dma_out(hist_bufs, dram, c):
                """Write segment chunk c (c >= 1; dram col base (c-1)*SLAB)."""
                src = hist_bufs[c % 2]
                cols = slice((c - 1) * SLAB, c * SLAB)
                for j in range(4):
                    nc.sync.dma_start(
                        dram[j * 128:(j + 1) * 128, cols],
                        src[:, j * SLAB:(j + 1) * SLAB])

            # --- main pipeline: L0 quarter s in slot s, L1 lags 2 slots ----
            # (lag 2 lets the u1 GEMMs drip between step matmuls a full slot
            # before they're consumed, instead of bunching at slot start)
            for op in u0_emit(0):
                op()
            for s in range(NQ + 2):
                pend = []
                if s + 1 < NQ:
                    pend += u0_emit(s + 1)
                if 1 <= s <= NQ:
                    pend += u1_emit(s - 1)
                for r in range(QT):
                    # HAM heater: zero-weight matmul streaming 512 cols keeps
                    # the PE activity monitor above the un-throttle threshold
                    # (2.4 GHz LDWEIGHTS). Adds +0 into the live u tile, so
                    # it is numerically inert; emitted BEFORE the step so its
                    # WAR dep (previous tanh) matches what the step matmuls
                    # already wait on — no added chain latency.
                    if r % 2 == 0:
                        ht = u0_tiles[s] if s < NQ else u1_tiles[s - 2]
                        nc.tensor.matmul(
                            ht[:, 0:512], lhsT=zero[:, :], rhs=w0[:, 0:512],
                            start=False, stop=False, skip_group_check=True)
                    if s < NQ:
                        step(0, s * QT + r)
                    if s >= 2:
                        step(1, (s - 2) * QT + r)
                    for _ in range(3):
                        if pend:
                            pend.pop(0)()
                while pend:
                    pend.pop(0)()
                if s >= 4 and s % 2 == 0:
                    dma_out(hist0, h0_d, s // 2 - 1)     # L0 chunks 1..8
                if s >= 7 and s % 2 == 1:
                    dma_out(hist1, h1_d, (s - 5) // 2)   # L1 chunks 1..7
            dma_out(hist1, h1_d, NC - 1)

    _split_sync_waits(nc)
    return nc


# ---------------------------------------------------------------------------
# host wrapper
# ---------------------------------------------------------------------------
def _prep_weight(w, scale):
    """[U,U] -> [128, 4*U] fp16 with block (k,m) at cols k*U + m*128."""
    a = (scale * w).astype(np.float16)
    return np.ascontiguousarray(
        a.reshape(4, 128, 4, 128).transpose(1, 0, 2, 3).reshape(128, 4 * U))


def kernel(x, kernel0, rec0, bias0, kernel1, rec1, bias1):
    _apply_patches()
    from concourse.bass_utils import run_bass_kernel_spmd

    x = np.asarray(x, dtype=np.float32)
    kernel0 = np.asarray(kernel0, dtype=np.float32)
    rec0 = np.asarray(rec0, dtype=np.float32)
    bias0 = np.asarray(bias0, dtype=np.float32)
    kernel1 = np.asarray(kernel1, dtype=np.float32)
    rec1 = np.asarray(rec1, dtype=np.float32)
    bias1 = np.asarray(bias1, dtype=np.float32)

    if "nc" not in _COMPILED:
        _COMPILED["nc"] = build_nc()
    nc = _COMPILED["nc"]

    in_maps = _make_in_maps(x, kernel0, rec0, bias0, kernel1, rec1, bias1)
    res = run_bass_kernel_spmd(nc, in_maps, list(range(NCORES)))

    out = np.empty((B, T, 2 * U), dtype=np.float32)
    for c in range(NCORES):
        for name, off in (("h0T", 0), ("h1T", U)):
            h = res.results[c][name].astype(np.float32) * ALPHA  # [U, SEG*B]
            out[:, c * SEG:(c + 1) * SEG, off:off + U] = (
                h.reshape(U, SEG, B).transpose(2, 1, 0))
    return out


def _make_in_maps(x, kernel0, rec0, bias0, kernel1, rec1, bias1):
    QS = QT * B
    k0 = kernel0.astype(np.float16)
    w0 = _prep_weight(rec0, ALPHA)
    k1 = _prep_weight(kernel1, ALPHA)
    w1 = _prep_weight(rec1, ALPHA)
    b0blk = np.ascontiguousarray(bias0.reshape(4, 128)).astype(np.float16)
    b1blk = np.ascontiguousarray(bias1.reshape(4, 128)).astype(np.float16)
    ind = np.zeros((4, 4 * QS), dtype=np.float16)
    for k in range(4):
        ind[k, k * QS:(k + 1) * QS] = 1.0
    TL = NC * Tc
    in_maps = []
    for c in range(NCORES):
        s = c * SEG
        xs = np.zeros((B, TL, I), dtype=np.float32)
        lo = max(s - LEAD, 0)
        xs[:, (lo - (s - LEAD)):] = x[:, lo:s + SEG]
        xT = np.ascontiguousarray(
            xs.transpose(2, 1, 0).reshape(I, TL * B)).astype(np.float16)
        mk = np.full((128, 4 * B), 0.0 if c == 0 else 1.0, dtype=np.float16)
        in_maps.append({
            "xT": xT, "k0": k0, "w0": w0, "k1": k1, "w1": w1,
            "b0blk": b0blk, "b1blk": b1blk, "ind": ind, "mask": mk,
        })
    return in_maps


def run_timed(x, kernel0, rec0, bias0, kernel1, rec1, bias1, tmpdir=None):
    """Run with NTFF profiling; returns BassKernelResults with exec_time_ns."""
    _apply_patches()
    import tempfile
    if tmpdir is None:
        tmpdir = tempfile.mkdtemp(prefix="dr_trace_")
    from concourse.bass_utils import run_bass_kernel_spmd
    if "nc" not in _COMPILED:
        _COMPILED["nc"] = build_nc()
    in_maps = _make_in_maps(
        np.asarray(x, np.float32), np.asarray(kernel0, np.float32),
        np.asarray(rec0, np.float32), np.asarray(bias0, np.float32),
        np.asarray(kernel1, np.float32), np.asarray(rec1, np.float32),
        np.asarray(bias1, np.float32))
    return run_bass_kernel_spmd(
        _COMPILED["nc"], in_maps, list(range(NCORES)), trace=True,
        tmpdir=tmpdir)


# revision 31
# speedup vs baseline: 1.0446x; 1.0446x over previous
"""DeepReservoir (2-layer leaky ESN) Trainium2 kernel — time-split, PSUM-u.

Reference computation (per layer):
    u = x @ K + b
    h_t = 0.1*h_{t-1} + 0.9*tanh(u_t + h_{t-1} @ W)
Layer 1 consumes layer 0's states; output = concat(s0, s1) on features.

Sharding: per-step cost is dominated by streaming the recurrent weights
into the PE array, independent of batch — so TIME-SPLIT, not batch
split: core c computes segment t in [c*128, (c+1)*128) for the full
batch (B=32), warm-started 32 steps earlier (the reservoir is
contractive; warm-up error ~1e-6). A per-core {0,1} mask zeroes the
state at the boundary so core 0's segment is exact from h=0.

Critical-path design: the projection u is accumulated IN PSUM — per
quarter (8 steps), a bias-seed matmul (rank-4: bias block times an
indicator) writes each PSUM bank once with start=True, the projection
GEMMs accumulate, and the per-step recurrent matmuls accumulate on
top (start=False; start=True clears a whole bank's has_written), so
tanh reads PSUM directly and the per-step chain is MM -> tanh -> leaky
update (no separate add). GEMM matmuls are drip-fed between step
matmuls to fill the PE stalls left by the chain latency. Layer 1 runs
one quarter behind layer 0. State stored scaled (htil = h/0.9, scales
folded into weights, final 0.9 rescale on host); matmuls fp16.
"""
import sys
import types

import numpy as np

B, T, I, U = 32, 1024, 64, 512     # full batch per core (time-split)
NCORES = 8
SEG = T // NCORES                  # 128 segment steps per core
LEAD = 16                          # warm-up steps (1 chunk)
Tc = 16                            # chunk length (state/DMA granularity)
QT = 8                             # steps per quarter (PSUM-u granularity)
NC = (LEAD + SEG) // Tc            # 9 chunks per core
NQ = NC * Tc // QT                 # 18 quarters
ALPHA = 0.9

_COMPILED = {}


# ---------------------------------------------------------------------------
# environment patches (inlined so kernel.py is self-contained)
# ---------------------------------------------------------------------------
def _apply_patches():
    import concourse.tile as tilemod
    from concourse.vector_clock import ScopedClock

    if not getattr(tilemod.TileContext, "_drain_patch_applied", False):
        def _drain_and_barrier(self, tick_clock, wait_clock):
            nc = self.nc
            drain_inst = nc.sync.drain()
            wait_clock.add_sem_waits(
                drain_inst.ins, ScopedClock({None: tick_clock.global_clock})
            )
            waits = list(drain_inst.ins.sync_info.on_wait)
            if len(waits) > 1:
                drain_inst.ins.sync_info.on_wait = waits[:1]
                for w in waits[1:]:
                    extra = nc.sync.drain()
                    si = extra.ins.sync_info
                    if si is None:
                        import bass_rust
                        extra.ins.sync_info = bass_rust.SyncInfo(
                            on_wait=[w], on_update=[]
                        )
                    else:
                        si.on_wait = [w]
            nc.all_engine_barrier()
            assert self.sems is not None
            popped = nc._tile_sem_poison_stack.pop()
            assert popped is self._sem_poison
            nc.clear_and_free_semaphores(list(self.sems.allocated().values()))
            nc.all_engine_barrier()

        tilemod.TileContext._drain_and_barrier = _drain_and_barrier
        tilemod.TileContext._drain_patch_applied = True

    import antenv
    if not hasattr(antenv, "axon_hooks"):
        mod = types.ModuleType("antenv.axon_hooks")
        mod._hook = None
        mod.set_axon_ntff_profile_hook = lambda h: setattr(mod, "_hook", h)
        mod.get_axon_ntff_profile_hook = lambda: mod._hook
        sys.modules["antenv.axon_hooks"] = mod
        antenv.axon_hooks = mod
        try:
            from trn_agent_boot.trn_boot import _ntff_profile_via_ctypes
            hook = _ntff_profile_via_ctypes("/opt/axon/libaxon_pjrt.so")
            if hook is not None:
                mod.set_axon_ntff_profile_hook(hook)
        except Exception:
            pass


def _split_sync_waits(nc, max_waits=1):
    """The public walrus rejects instructions with >2 sync-wait commands.
    Spread overflow waits onto same-engine NOPs inserted just before."""
    import concourse.mybir as mybir

    for f in nc.m.functions:
        for blk in f.blocks:
            insts = blk.instructions
            out = []
            changed = False
            for inst in insts:
                si = getattr(inst, "sync_info", None)
                waits = list(si.on_wait) if si is not None else []
                if len(waits) > max_waits:
                    changed = True
                    overflow = waits[:-max_waits]
                    si.on_wait = waits[-max_waits:]
                    for i in range(0, len(overflow), max_waits):
                        nop = mybir.InstNoOp(
                            name=nc.get_next_instruction_name(),
                            sync_info=mybir.SyncInfo(
                                on_wait=overflow[i:i + max_waits], on_update=[]
                            ),
                            bass_nofuse=True,
                            engine=inst.engine,
                        )
                        out.append(nop)
                out.append(inst)
            if changed:
                blk.instructions = out
    return nc


# ---------------------------------------------------------------------------
# kernel builder
# ---------------------------------------------------------------------------
def build_nc():
    import concourse.bass as bass
    import concourse.tile as tile
    import concourse.mybir as mybir

    f32 = mybir.dt.float32
    f16 = mybir.dt.float16
    S = 4 * B                  # free cols per step (4 U-tiles x B) = 128
    SLAB = Tc * B              # cols per (chunk, feature-tile) slab = 1024
    QS = QT * B                # cols per (quarter, feature-tile) slab = 256
    TL = NC * Tc               # local timeline length = 160

    nc = bass.Bass(trn_type="TRN2")

    xT_d = nc.declare_dram_parameter("xT", (I, TL * B), f16, isOutput=False)
    k0_d = nc.declare_dram_parameter("k0", (I, U), f16, isOutput=False)
    w0_d = nc.declare_dram_parameter("w0", (128, 4 * U), f16, isOutput=False)
    k1_d = nc.declare_dram_parameter("k1", (128, 4 * U), f16, isOutput=False)
    w1_d = nc.declare_dram_parameter("w1", (128, 4 * U), f16, isOutput=False)
    b0_d = nc.declare_dram_parameter("b0blk", (4, 128), f16, isOutput=False)
    bb_d = nc.declare_dram_parameter("b1blk", (4, 128), f16, isOutput=False)
    in_d = nc.declare_dram_parameter("ind", (4, 4 * QS), f16, isOutput=False)
    mk_d = nc.declare_dram_parameter("mask", (128, S), f16, isOutput=False)
    h0_d = nc.declare_dram_parameter("h0T", (U, SEG * B), f16, isOutput=True)
    h1_d = nc.declare_dram_parameter("h1T", (U, SEG * B), f16, isOutput=True)

    with tile.TileContext(nc) as tc:
        import contextlib
        with contextlib.ExitStack() as ctx:
            const = ctx.enter_context(tc.tile_pool(name="const", bufs=1))
            ypool = ctx.enter_context(tc.tile_pool(name="ypool", bufs=3))
            up0 = ctx.enter_context(
                tc.tile_pool(name="up0", bufs=2, space="PSUM"))
            up1 = ctx.enter_context(
                tc.tile_pool(name="up1", bufs=2, space="PSUM"))

            # --- resident constants: startup DMAs spread across engine DGE
            # queues so they run in parallel rings; the critical first-step
            # set (k0/b0blk/ind/w0/xT head) gets its own queues ------------
            k0 = const.tile([I, U], f16, tag="k0")
            nc.sync.dma_start(k0[:], k0_d[:, :])
            b0blk = const.tile([4, 128], f16, tag="b0blk")
            nc.sync.dma_start(b0blk[:], b0_d[:, :])
            ind = const.tile([4, 4 * QS], f16, tag="ind")
            nc.sync.dma_start(ind[:], in_d[:, :])
            w0 = const.tile([128, 4 * U], f16, tag="w0")
            nc.scalar.dma_start(w0[:, :2 * U], w0_d[:, :2 * U])
            nc.gpsimd.dma_start(w0[:, 2 * U:], w0_d[:, 2 * U:])
            xT = const.tile([I, TL * B], f16, tag="xT")
            XSPLIT = 4 * QS
            nc.sync.dma_start(xT[:, :XSPLIT], xT_d[:, :XSPLIT])
            b1blk = const.tile([4, 128], f16, tag="b1blk")
            nc.sync.dma_start(b1blk[:], bb_d[:, :])
            k1 = const.tile([128, 4 * U], f16, tag="k1")
            nc.scalar.dma_start(k1[:], k1_d[:, :])
            w1 = const.tile([128, 4 * U], f16, tag="w1")
            nc.gpsimd.dma_start(w1[:], w1_d[:, :])
            mask = const.tile([128, S], f16, tag="mask")
            nc.sync.dma_start(mask[:], mk_d[:, :])
            nc.sync.dma_start(xT[:, XSPLIT:], xT_d[:, XSPLIT:])
            zero = const.tile([128, S], f16, tag="zero")
            nc.vector.memset(zero[:], 0.0)

            # state chunks, double-buffered (slab layout:
            # col = j*SLAB + t*B + b, j = feature 128-tile)
            hist0 = [const.tile([128, Tc * S], f16, tag=f"hist0_{i}", name=f"hist0_{i}") for i in range(2)]
            hist1 = [const.tile([128, Tc * S], f16, tag=f"hist1_{i}", name=f"hist1_{i}") for i in range(2)]

            # per-quarter PSUM u tiles (col = m*QS + r*B + b), keyed by quarter
            u0_tiles = {}
            u1_tiles = {}

            def mm_rhs_ap(hist_bufs, t, j):
                """[128, B] moving operand: state k-tile j at step t."""
                if t < 0:
                    return zero[:, j * B:(j + 1) * B]
                return hist_bufs[(t // Tc) % 2][
                    :, j * SLAB + (t % Tc) * B: j * SLAB + (t % Tc + 1) * B]

            def state_xs(hist_bufs, t):
                """[128, 4, B] strided cross-section of the state at step t."""
                if t < 0:
                    return zero[:, :].rearrange("p (j b) -> p j b", j=4)
                buf = hist_bufs[(t // Tc) % 2]
                r = t % Tc
                return buf[:, :].rearrange(
                    "p (j tb) -> p j tb", j=4)[:, :, r * B:(r + 1) * B]

            def u0_emit(q):
                """Closures: u0 quarter q = bias seed + x-projection.
                Each PSUM bank gets exactly ONE start=True write (the seed);
                start=True clears the whole bank's has_written bits, so any
                later start on a bank would break accumulation into it."""
                tl = up0.tile([128, QT * S], f32, tag="u0q", name="u0q")
                u0_tiles[q] = tl
                cols = slice(q * QS, (q + 1) * QS)
                ops = [
                    (lambda hh=hh, tl=tl: nc.tensor.matmul(
                        tl[:, hh * 512:(hh + 1) * 512], lhsT=b0blk[:, :],
                        rhs=ind[:, hh * 512:(hh + 1) * 512],
                        start=True, stop=False, skip_group_check=True))
                    for hh in range(2)
                ]
                ops += [
                    (lambda m=m, tl=tl: nc.tensor.matmul(
                        tl[:, m * QS:(m + 1) * QS],
                        lhsT=k0[:, m * 128:(m + 1) * 128], rhs=xT[:, cols],
                        start=False, stop=False, skip_group_check=True))
                    for m in range(4)
                ]
                return ops

            def u1_emit(q):
                """Closures: u1 quarter q = bias seed + hist0-projection."""
                tl = up1.tile([128, QT * S], f32, tag="u1q", name="u1q")
                u1_tiles[q] = tl
                src = hist0[(q * QT // Tc) % 2]
                qo = (q % (Tc // QT)) * QS
                ops = [
                    (lambda hh=hh, tl=tl: nc.tensor.matmul(
                        tl[:, hh * 512:(hh + 1) * 512], lhsT=b1blk[:, :],
                        rhs=ind[:, hh * 512:(hh + 1) * 512],
                        start=True, stop=False, skip_group_check=True))
                    for hh in range(2)
                ]
                for m in range(4):
                    for k in range(4):
                        ops.append(lambda m=m, k=k, tl=tl: nc.tensor.matmul(
                            tl[:, m * QS:(m + 1) * QS],
                            lhsT=k1[:, k * U + m * 128: k * U + (m + 1) * 128],
                            rhs=src[:, k * SLAB + qo: k * SLAB + qo + QS],
                            start=False, stop=False, skip_group_check=True))
                return ops

            def step(layer, t):
                w, utiles, hist = (
                    (w0, u0_tiles, hist0) if layer == 0 else
                    (w1, u1_tiles, hist1))
                tl = utiles[t // QT]
                r8 = t % QT
                for m in range(4):
                    for k in range(4):
                        nc.tensor.matmul(
                            tl[:, m * QS + r8 * B: m * QS + (r8 + 1) * B],
                            lhsT=w[:, k * U + m * 128: k * U + (m + 1) * 128],
                            rhs=mm_rhs_ap(hist, t - 1, k),
                            start=False, stop=False, skip_group_check=True)
                u_xs = tl[:, :].rearrange(
                    "p (m tb) -> p m tb", m=4)[:, :, r8 * B:(r8 + 1) * B]
                y = ypool.tile([128, S], f16, tag="y", name="y")
                nc.scalar.activation(
                    y[:].rearrange("p (m b) -> p m b", m=4), u_xs,
                    mybir.ActivationFunctionType.Tanh)
                nc.vector.scalar_tensor_tensor(
                    state_xs(hist, t), state_xs(hist, t - 1), 0.1,
                    y[:].rearrange("p (j b) -> p j b", j=4),
                    op0=mybir.AluOpType.mult, op1=mybir.AluOpType.add)
                if t == LEAD - 1:
                    # warm-up/segment boundary: core 0 (mask=0) starts its
                    # segment from the exact h=0 initial state
                    nc.vector.scalar_tensor_tensor(
                        state_xs(hist, t), state_xs(hist, t), 1.0,
                        mask[:, :].rearrange("p (j b) -> p j b", j=4),
                        op0=mybir.AluOpType.mult, op1=mybir.AluOpType.mult)

            def dma_out(hist_bufs, dram, c):
                """Write segment chunk c (c >= 1; dram col base (c-1)*SLAB)."""
                src = hist_bufs[c % 2]
                cols = slice((c - 1) * SLAB, c * SLAB)
                for j in range(4):
                    nc.sync.dma_start(
                        dram[j * 128:(j + 1) * 128, cols],
                        src[:, j * SLAB:(j + 1) * SLAB])

            # --- main pipeline: L0 quarter s in slot s, L1 lags 2 slots ----
            # (lag 2 lets the u1 GEMMs drip between step matmuls a full slot
            # before they're consumed, instead of bunching at slot start)
            for op in u0_emit(0):
                op()
            for s in range(NQ + 2):
                pend = []
                if s + 1 < NQ:
                    pend += u0_emit(s + 1)
                if 1 <= s <= NQ:
                    pend += u1_emit(s - 1)
                for r in range(QT):
                    # HAM heater: zero-weight matmul streaming 512 cols keeps
                    # the PE activity monitor above the un-throttle threshold
                    # (2.4 GHz LDWEIGHTS). Adds +0 into the live u tile, so
                    # it is numerically inert; emitted BEFORE the step so its
                    # WAR dep (previous tanh) matches what the step matmuls
                    # already wait on — no added chain latency.
                    ht = u0_tiles[s] if s < NQ else u1_tiles[s - 2]
                    nc.tensor.matmul(
                        ht[:, 0:512], lhsT=zero[:, :], rhs=w0[:, 0:512],
                        start=False, stop=False, skip_group_check=True)
                    if s < NQ:
                        step(0, s * QT + r)
                    if s >= 2:
                        step(1, (s - 2) * QT + r)
                    for _ in range(3):
                        if pend:
                            pend.pop(0)()
                while pend:
                    pend.pop(0)()
                if s >= 4 and s % 2 == 0:
                    dma_out(hist0, h0_d, s // 2 - 1)     # L0 chunks 1..8
                if s >= 7 and s % 2 == 1:
                    dma_out(hist1, h1_d, (s - 5) // 2)   # L1 chunks 1..7
            dma_out(hist1, h1_d, NC - 1)

    _split_sync_waits(nc)
    return nc


# ---------------------------------------------------------------------------
# host wrapper
# ---------------------------------------------------------------------------
def _prep_weight(w, scale):
    """[U,U] -> [128, 4*U] fp16 with block (k,m) at cols k*U + m*128."""
    a = (scale * w).astype(np.float16)
    return np.ascontiguousarray(
        a.reshape(4, 128, 4, 128).transpose(1, 0, 2, 3).reshape(128, 4 * U))


def kernel(x, kernel0, rec0, bias0, kernel1, rec1, bias1):
    _apply_patches()
    from concourse.bass_utils import run_bass_kernel_spmd

    x = np.asarray(x, dtype=np.float32)
    kernel0 = np.asarray(kernel0, dtype=np.float32)
    rec0 = np.asarray(rec0, dtype=np.float32)
    bias0 = np.asarray(bias0, dtype=np.float32)
    kernel1 = np.asarray(kernel1, dtype=np.float32)
    rec1 = np.asarray(rec1, dtype=np.float32)
    bias1 = np.asarray(bias1, dtype=np.float32)

    if "nc" not in _COMPILED:
        _COMPILED["nc"] = build_nc()
    nc = _COMPILED["nc"]

    in_maps = _make_in_maps(x, kernel0, rec0, bias0, kernel1, rec1, bias1)
    res = run_bass_kernel_spmd(nc, in_maps, list(range(NCORES)))

    out = np.empty((B, T, 2 * U), dtype=np.float32)
    for c in range(NCORES):
        for name, off in (("h0T", 0), ("h1T", U)):
            h = res.results[c][name].astype(np.float32) * ALPHA  # [U, SEG*B]
            out[:, c * SEG:(c + 1) * SEG, off:off + U] = (
                h.reshape(U, SEG, B).transpose(2, 1, 0))
    return out


def _make_in_maps(x, kernel0, rec0, bias0, kernel1, rec1, bias1):
    QS = QT * B
    k0 = kernel0.astype(np.float16)
    w0 = _prep_weight(rec0, ALPHA)
    k1 = _prep_weight(kernel1, ALPHA)
    w1 = _prep_weight(rec1, ALPHA)
    b0blk = np.ascontiguousarray(bias0.reshape(4, 128)).astype(np.float16)
    b1blk = np.ascontiguousarray(bias1.reshape(4, 128)).astype(np.float16)
    ind = np.zeros((4, 4 * QS), dtype=np.float16)
    for k in range(4):
        ind[k, k * QS:(k + 1) * QS] = 1.0
    TL = NC * Tc
    in_maps = []
    for c in range(NCORES):
        s = c * SEG
        xs = np.zeros((B, TL, I), dtype=np.float32)
        lo = max(s - LEAD, 0)
        xs[:, (lo - (s - LEAD)):] = x[:, lo:s + SEG]
        xT = np.ascontiguousarray(
            xs.transpose(2, 1, 0).reshape(I, TL * B)).astype(np.float16)
        mk = np.full((128, 4 * B), 0.0 if c == 0 else 1.0, dtype=np.float16)
        in_maps.append({
            "xT": xT, "k0": k0, "w0": w0, "k1": k1, "w1": w1,
            "b0blk": b0blk, "b1blk": b1blk, "ind": ind, "mask": mk,
        })
    return in_maps


def run_timed(x, kernel0, rec0, bias0, kernel1, rec1, bias1, tmpdir=None):
    """Run with NTFF profiling; returns BassKernelResults with exec_time_ns."""
    _apply_patches()
    import tempfile
    if tmpdir is None:
        tmpdir = tempfile.mkdtemp(prefix="dr_trace_")
    from concourse.bass_utils import run_bass_kernel_spmd
    if "nc" not in _COMPILED:
        _COMPILED["nc"] = build_nc()
    in_maps = _make_in_maps(
        np.asarray(x, np.float32), np.asarray(kernel0, np.float32),
        np.asarray(rec0, np.float32), np.asarray(bias0, np.float32),
        np.asarray(kernel1, np.float32), np.asarray(rec1, np.float32),
        np.asarray(bias1, np.float32))
    return run_bass_kernel_spmd(
        _COMPILED["nc"], in_maps, list(range(NCORES)), trace=True,
        tmpdir=tmpdir)
